# revision 24
# baseline (speedup 1.0000x reference)
"""Sharded multi-head attention for TRN2 (8 NeuronCores).

Problem: B=4, H=16, S=2048, DK=64 attention with boolean mask [B,1,S,S]
(True entries masked out).  The 64 (batch, head) pairs are independent:
core c handles batch c//2, heads (c%2)*8 .. (c%2)*8+8.

Per-core algorithm, heads processed in PAIRS (A, B) sharing the PE array:
  - scores_T[k, q] for A and B run CONCURRENTLY in PE row-groups [0:64] /
    [64:128] (contraction dim d=64 each, tile_position packing).
  - one ACT exp over the pair's [128, 1024] PSUM tile (scale=1/8 folded,
    no max-subtraction: scores ~ N(0,1)).
  - mask multiply on DVE (bf16 2x), keep_T shared across heads.
  - PV: acc[v, q] += V'[k, v]^T w per head, V' = [V | ones] so row 64 of
    acc accumulates the softmax denominators.
  - epilogue per (head, q-chunk): reciprocal_approx_fast of row 64,
    rank-1 PE broadcast to 64 partitions, multiply, DMA out in [d, q]
    layout (host un-transposes the view; pure layout, no host math).

All DMAs are partition-major with >=2KB contiguous runs (host pre-swizzles
inputs, ones column baked into V').
"""

import numpy as np
import ml_dtypes
from contextlib import ExitStack

import concourse.bass as bass
import concourse.tile as tile
from concourse import bacc, mybir
from concourse.bass_utils import run_bass_kernel_spmd

B, H, S, DK = 4, 16, 2048, 64
N_CORES = 8
HPC = (B * H) // N_CORES  # heads per core = 8
NPAIR = HPC // 2

P = 128            # k-tile size / partition count
NKT = S // P       # 16 k tiles
QCH = 512          # q chunk per head (pair tile = [128, 1024] = 2 PSUM banks)
NQ = S // QCH      # 4 q chunks

BF16 = mybir.dt.bfloat16
F32 = mybir.dt.float32
BF = ml_dtypes.bfloat16


def build_nc():
    nc = bacc.Bacc(None, target_bir_lowering=False)
    # qkt[pair, 0] = [Q_A^T ; Q_B^T] stacked on partitions, [pair, 1] = K
    qkt_ext = nc.declare_dram_parameter("qkt", [NPAIR, 2, P, S], BF16, isOutput=False)
    # vp[h, p, t, :] = [V[h, t*128+p, :], 1.0]
    vp_ext = nc.declare_dram_parameter("vp", [HPC, P, NKT, DK + 1], BF16, isOutput=False)
    # keep[p, t, q] = not mask[q, t*128+p]
    keep_ext = nc.declare_dram_parameter("keep", [P, NKT, S], BF16, isOutput=False)
    # out_T[h, d, q] (host un-transposes)
    out_ext = nc.declare_dram_parameter("outT", [HPC, DK, S], F32, isOutput=True)

    with tile.TileContext(nc) as tc, ExitStack() as ctx:
        singles = ctx.enter_context(tc.tile_pool(name="singles", bufs=1))
        qk_pool = ctx.enter_context(tc.tile_pool(name="qk", bufs=2))
        v_pool = ctx.enter_context(tc.tile_pool(name="vpool", bufs=2))
        w_pool = ctx.enter_context(tc.tile_pool(name="wp", bufs=4))
        ep_pool = ctx.enter_context(tc.tile_pool(name="ep", bufs=2))
        sc_ps = ctx.enter_context(tc.tile_pool(name="scps", bufs=2, space="PSUM"))
        acc_ps = ctx.enter_context(tc.tile_pool(name="accps", bufs=2, space="PSUM"))

        keep_sb = singles.tile([P, NKT, S], BF16)
        for kt in range(NKT):
            nc.gpsimd.dma_start(out=keep_sb[:, kt], in_=keep_ext[:, kt])

        # ---- flat software pipeline over (pair, qc, kt) ----
        pair_tiles = {}

        def get_pair(pair):
            if pair not in pair_tiles:
                hA, hB = 2 * pair, 2 * pair + 1
                qT2 = qk_pool.tile([P, S], BF16, tag="qT2", name=f"qT2_{pair}")
                kT2 = qk_pool.tile([P, S], BF16, tag="kT2", name=f"kT2_{pair}")
                nc.sync.dma_start(out=qT2, in_=qkt_ext[pair, 0])
                nc.sync.dma_start(out=kT2, in_=qkt_ext[pair, 1])
                vpA = v_pool.tile([P, NKT, DK + 1], BF16, tag="vpA", name=f"vpA_{pair}")
                vpB = v_pool.tile([P, NKT, DK + 1], BF16, tag="vpB", name=f"vpB_{pair}")
                nc.sync.dma_start(out=vpA, in_=vp_ext[hA])
                nc.sync.dma_start(out=vpB, in_=vp_ext[hB])
                pair_tiles[pair] = (qT2, kT2, vpA, vpB)
            return pair_tiles[pair]

        def issue_qk(pair, qc, kt):
            qT2, kT2, _, _ = get_pair(pair)
            q0, k0 = qc * QCH, kt * P
            sc = sc_ps.tile([P, 2 * QCH], F32, tag="sc", name=f"sc_{pair}_{qc}_{kt}")
            nc.tensor.matmul(
                sc[:, 0:QCH],
                kT2[0:DK, k0 : k0 + P],
                qT2[0:DK, q0 : q0 + QCH],
                start=True,
                stop=True,
                tile_position=(0, 0),
            )
            nc.tensor.matmul(
                sc[:, QCH : 2 * QCH],
                kT2[DK : 2 * DK, k0 : k0 + P],
                qT2[DK : 2 * DK, q0 : q0 + QCH],
                start=True,
                stop=True,
                tile_position=(64, 0),
            )
            return sc

        def ep_recip(tag, acc):
            """reciprocal of the sums row (DVE, reads PSUM)"""
            rowF = ep_pool.tile([1, QCH], F32, tag=f"rowF{tag}")
            nc.vector.tensor_copy(rowF, acc[DK : DK + 1, :])
            recipF = ep_pool.tile([1, QCH], F32, tag=f"recipF{tag}")
            nc.vector.reciprocal_approx_fast(recipF, rowF)
            recipS = ep_pool.tile([1, QCH], BF16, tag=f"recipS{tag}")
            nc.vector.tensor_copy(recipS, recipF)
            bcS = ep_pool.tile([DK, QCH], BF16, tag=f"bcS{tag}")
            nc.gpsimd.partition_broadcast(bcS, recipS)
            return bcS

        def ep_copy(tag, acc):
            """PSUM -> SBUF copy of the accumulator (DVE)"""
            accS = ep_pool.tile([DK + 1, QCH], BF16, tag=f"accS{tag}")
            nc.vector.tensor_copy(accS, acc)
            return accS

        def ep_store(h, qc, accS, bcS):
            """normalize on GpSimd (SBUF-only operands) + store"""
            q0 = qc * QCH
            outf = ep_pool.tile([DK, QCH], F32, tag="outf")
            nc.gpsimd.tensor_mul(outf, accS[0:DK], bcS)
            nc.gpsimd.dma_start(out=out_ext[h, :, q0 : q0 + QCH], in_=outf)

        iters = [
            (pr, qc, kt)
            for pr in range(NPAIR)
            for qc in range(NQ)
            for kt in range(NKT)
        ]
        # epilogue micro-ops staggered one-per-iteration so the DVE FIFO
        # never carries a burst; (due_iter, fn) closures
        pending_ops = []
        accs = None
        sc = issue_qk(*iters[0])
        for i, (pr, qc, kt) in enumerate(iters):
            if kt == 0:
                accs = (
                    acc_ps.tile([DK + 1, QCH], F32, tag="accA", name=f"accA_{pr}_{qc}"),
                    acc_ps.tile([DK + 1, QCH], F32, tag="accB", name=f"accB_{pr}_{qc}"),
                )
            accA, accB = accs
            w = w_pool.tile([P, 2 * QCH], BF16, tag="w")
            nc.scalar.activation(w, sc, mybir.ActivationFunctionType.Exp, scale=0.125)
            # next iteration's QK issues BEFORE this iteration's PV so the
            # in-order PE queue never parks QK behind a PV still waiting on
            # exp/mask; crosses qc/pair boundaries
            if i + 1 < len(iters):
                sc = issue_qk(*iters[i + 1])
            # one masked multiply over both heads: keep slice broadcast
            # (stride-0) over the head dim
            q0 = qc * QCH
            keep_slice = keep_sb[:, kt, q0 : q0 + QCH]
            keep2 = bass.AP(
                tensor=keep_slice.tensor,
                offset=keep_slice.offset,
                ap=[keep_slice.ap[0], [0, 2], keep_slice.ap[1]],
            )
            w2 = w.rearrange("p (r q) -> p r q", r=2)
            nc.vector.tensor_mul(w2, w2, keep2)
            vpA, vpB = pair_tiles[pr][2], pair_tiles[pr][3]
            nc.tensor.matmul(
                accA, vpA[:, kt], w[:, 0:QCH],
                start=(kt == 0), stop=(kt == NKT - 1),
            )
            nc.tensor.matmul(
                accB, vpB[:, kt], w[:, QCH : 2 * QCH],
                start=(kt == 0), stop=(kt == NKT - 1),
            )
            if kt == NKT - 1:
                # build the staggered epilogue schedule for this (pr, qc)
                state = {}
                hA2, hB2, eqc = 2 * pr, 2 * pr + 1, qc

                def mk(fn):
                    return fn

                pending_ops += [
                    (i + 2, mk(lambda s=state, a=accA: s.__setitem__("bcA", ep_recip("A", a)))),
                    (i + 3, mk(lambda s=state, a=accB: s.__setitem__("bcB", ep_recip("B", a)))),
                    (i + 5, mk(lambda s=state, a=accA: s.__setitem__("aA", ep_copy("A", a)))),
                    (i + 6, mk(lambda s=state, h=hA2, q=eqc: ep_store(h, q, s["aA"], s["bcA"]))),
                    (i + 7, mk(lambda s=state, a=accB: s.__setitem__("aB", ep_copy("B", a)))),
                    (i + 8, mk(lambda s=state, h=hB2, q=eqc: ep_store(h, q, s["aB"], s["bcB"]))),
                ]
            while pending_ops and pending_ops[0][0] <= i:
                pending_ops.pop(0)[1]()
        for _, fn in pending_ops:
            fn()
    nc.finalize()
    return nc


_NC_CACHE = {}


def get_nc():
    if "nc" not in _NC_CACHE:
        _NC_CACHE["nc"] = build_nc()
    return _NC_CACHE["nc"]


def kernel(Q, K, V, mask, _trace=False, _tmpdir=None):
    Q = np.asarray(Q, dtype=np.float32)
    K = np.asarray(K, dtype=np.float32)
    V = np.asarray(V, dtype=np.float32)
    mask = np.asarray(mask)

    in_maps = []
    for c in range(N_CORES):
        b, h0 = c // 2, (c % 2) * HPC
        # [pair, {q,k}, 128, S]: partitions 0:64 = head A dims, 64:128 = head B
        qkt = np.empty((NPAIR, 2, P, S), BF)
        qt = Q[b, h0 : h0 + HPC].transpose(0, 2, 1).reshape(NPAIR, 2 * DK, S)
        kt = K[b, h0 : h0 + HPC].transpose(0, 2, 1).reshape(NPAIR, 2 * DK, S)
        qkt[:, 0] = qt
        qkt[:, 1] = kt
        vp = np.empty((HPC, P, NKT, DK + 1), BF)
        vp[:, :, :, 0:DK] = (
            V[b, h0 : h0 + HPC].reshape(HPC, NKT, P, DK).transpose(0, 2, 1, 3)
        )
        vp[:, :, :, DK] = 1.0
        if c % 2 == 0:
            kp = (~mask[b, 0]).T  # [k, q]
            keep = np.ascontiguousarray(
                kp.reshape(NKT, P, S).transpose(1, 0, 2)
            ).astype(BF)
        in_maps.append({"qkt": qkt, "vp": vp, "keep": keep})

    nc = get_nc()
    res = run_bass_kernel_spmd(
        nc, in_maps, core_ids=list(range(N_CORES)), trace=_trace, tmpdir=_tmpdir
    )
    out = np.empty((B, H, S, DK), np.float32)
    for c in range(N_CORES):
        b, h0 = c // 2, (c % 2) * HPC
        out[b, h0 : h0 + HPC] = np.asarray(res.results[c]["outT"]).transpose(0, 2, 1)
    if _trace:
        return out, res
    return out


# revision 25
# speedup vs baseline: 1.0795x; 1.0795x over previous
"""Sharded multi-head attention for TRN2 (8 NeuronCores).

Problem: B=4, H=16, S=2048, DK=64 attention with boolean mask [B,1,S,S]
(True entries masked out).  The 64 (batch, head) pairs are independent:
core c handles batch c//2, heads (c%2)*8 .. (c%2)*8+8.

Per-core algorithm, heads processed in PAIRS (A, B) sharing the PE array:
  - scores_T[k, q] for A and B run CONCURRENTLY in PE row-groups [0:64] /
    [64:128] (contraction dim d=64 each, tile_position packing).
  - one ACT exp over the pair's [128, 1024] PSUM tile (scale=1/8 folded,
    no max-subtraction: scores ~ N(0,1)).
  - mask multiply on DVE (bf16 2x), keep_T shared across heads.
  - PV: acc[v, q] += V'[k, v]^T w per head, V' = [V | ones] so row 64 of
    acc accumulates the softmax denominators.
  - epilogue per (head, q-chunk): reciprocal_approx_fast of row 64,
    rank-1 PE broadcast to 64 partitions, multiply, DMA out in [d, q]
    layout (host un-transposes the view; pure layout, no host math).

All DMAs are partition-major with >=2KB contiguous runs (host pre-swizzles
inputs, ones column baked into V').
"""

import numpy as np
import ml_dtypes
from contextlib import ExitStack

import concourse.bass as bass
import concourse.tile as tile
from concourse import bacc, mybir
from concourse.bass_utils import run_bass_kernel_spmd

B, H, S, DK = 4, 16, 2048, 64
N_CORES = 8
HPC = (B * H) // N_CORES  # heads per core = 8
NPAIR = HPC // 2

P = 128            # k-tile size / partition count
NKT = S // P       # 16 k tiles
QCH = 512          # q chunk per head (pair tile = [128, 1024] = 2 PSUM banks)
NQ = S // QCH      # 4 q chunks

BF16 = mybir.dt.bfloat16
F32 = mybir.dt.float32
BF = ml_dtypes.bfloat16


def build_nc():
    nc = bacc.Bacc(None, target_bir_lowering=False)
    # qkt[pair, 0] = [Q_A^T ; Q_B^T] stacked on partitions, [pair, 1] = K
    qkt_ext = nc.declare_dram_parameter("qkt", [NPAIR, 2, P, S], BF16, isOutput=False)
    # vp[h, p, t, :] = [V[h, t*128+p, :], 1.0]
    vp_ext = nc.declare_dram_parameter("vp", [HPC, P, NKT, DK + 1], BF16, isOutput=False)
    # keep[p, t, q] = not mask[q, t*128+p]
    keep_ext = nc.declare_dram_parameter("keep", [P, NKT, S], BF16, isOutput=False)
    # out_T[h, d, q] (host un-transposes)
    out_ext = nc.declare_dram_parameter("outT", [HPC, DK, S], F32, isOutput=True)

    with tile.TileContext(nc) as tc, ExitStack() as ctx:
        singles = ctx.enter_context(tc.tile_pool(name="singles", bufs=1))
        qk_pool = ctx.enter_context(tc.tile_pool(name="qk", bufs=2))
        v_pool = ctx.enter_context(tc.tile_pool(name="vpool", bufs=2))
        w_pool = ctx.enter_context(tc.tile_pool(name="wp", bufs=4))
        ep_pool = ctx.enter_context(tc.tile_pool(name="ep", bufs=2))
        sc_ps = ctx.enter_context(tc.tile_pool(name="scps", bufs=2, space="PSUM"))
        acc_ps = ctx.enter_context(tc.tile_pool(name="accps", bufs=2, space="PSUM"))

        keep_sb = singles.tile([P, NKT, S], BF16)
        for kt in range(NKT):
            nc.gpsimd.dma_start(out=keep_sb[:, kt], in_=keep_ext[:, kt])

        # ---- flat software pipeline over (pair, qc, kt) ----
        pair_tiles = {}

        def get_pair(pair):
            if pair not in pair_tiles:
                hA, hB = 2 * pair, 2 * pair + 1
                qT2 = qk_pool.tile([P, S], BF16, tag="qT2", name=f"qT2_{pair}")
                kT2 = qk_pool.tile([P, S], BF16, tag="kT2", name=f"kT2_{pair}")
                nc.sync.dma_start(out=qT2, in_=qkt_ext[pair, 0])
                nc.sync.dma_start(out=kT2, in_=qkt_ext[pair, 1])
                vpA = v_pool.tile([P, NKT, DK + 1], BF16, tag="vpA", name=f"vpA_{pair}")
                vpB = v_pool.tile([P, NKT, DK + 1], BF16, tag="vpB", name=f"vpB_{pair}")
                nc.sync.dma_start(out=vpA, in_=vp_ext[hA])
                nc.sync.dma_start(out=vpB, in_=vp_ext[hB])
                pair_tiles[pair] = (qT2, kT2, vpA, vpB)
            return pair_tiles[pair]

        def issue_qk(pair, qc, kt):
            qT2, kT2, _, _ = get_pair(pair)
            q0, k0 = qc * QCH, kt * P
            sc = sc_ps.tile([P, 2 * QCH], F32, tag="sc", name=f"sc_{pair}_{qc}_{kt}")
            nc.tensor.matmul(
                sc[:, 0:QCH],
                kT2[0:DK, k0 : k0 + P],
                qT2[0:DK, q0 : q0 + QCH],
                start=True,
                stop=True,
                tile_position=(0, 0),
            )
            nc.tensor.matmul(
                sc[:, QCH : 2 * QCH],
                kT2[DK : 2 * DK, k0 : k0 + P],
                qT2[DK : 2 * DK, q0 : q0 + QCH],
                start=True,
                stop=True,
                tile_position=(64, 0),
            )
            return sc

        def ep_recip(tag, acc):
            """reciprocal of the sums row (DVE, reads PSUM)"""
            rowF = ep_pool.tile([1, QCH], F32, tag=f"rowF{tag}")
            nc.vector.tensor_copy(rowF, acc[DK : DK + 1, :])
            recipF = ep_pool.tile([1, QCH], F32, tag=f"recipF{tag}")
            nc.vector.reciprocal_approx_fast(recipF, rowF)
            recipS = ep_pool.tile([1, QCH], BF16, tag=f"recipS{tag}")
            nc.vector.tensor_copy(recipS, recipF)
            bcS = ep_pool.tile([DK, QCH], BF16, tag=f"bcS{tag}")
            nc.gpsimd.partition_broadcast(bcS, recipS)
            return bcS

        def ep_copy(tag, acc):
            """PSUM -> SBUF copy of the accumulator (DVE)"""
            accS = ep_pool.tile([DK + 1, QCH], BF16, tag=f"accS{tag}")
            nc.vector.tensor_copy(accS, acc)
            return accS

        def ep_store(h, qc, accS, bcS):
            """normalize + store"""
            q0 = qc * QCH
            outf = ep_pool.tile([DK, QCH], F32, tag="outf")
            nc.vector.tensor_mul(outf, accS[0:DK], bcS)
            nc.gpsimd.dma_start(out=out_ext[h, :, q0 : q0 + QCH], in_=outf)

        iters = [
            (pr, qc, kt)
            for pr in range(NPAIR)
            for qc in range(NQ)
            for kt in range(NKT)
        ]
        # epilogue micro-ops staggered one-per-iteration so the DVE FIFO
        # never carries a burst; (due_iter, fn) closures
        pending_ops = []
        accs = None
        sc = issue_qk(*iters[0])
        for i, (pr, qc, kt) in enumerate(iters):
            if kt == 0:
                accs = (
                    acc_ps.tile([DK + 1, QCH], F32, tag="accA", name=f"accA_{pr}_{qc}"),
                    acc_ps.tile([DK + 1, QCH], F32, tag="accB", name=f"accB_{pr}_{qc}"),
                )
            accA, accB = accs
            w = w_pool.tile([P, 2 * QCH], BF16, tag="w")
            nc.scalar.activation(w, sc, mybir.ActivationFunctionType.Exp, scale=0.125)
            # next iteration's QK issues BEFORE this iteration's PV so the
            # in-order PE queue never parks QK behind a PV still waiting on
            # exp/mask; crosses qc/pair boundaries
            if i + 1 < len(iters):
                sc = issue_qk(*iters[i + 1])
            # one masked multiply over both heads: keep slice broadcast
            # (stride-0) over the head dim
            q0 = qc * QCH
            keep_slice = keep_sb[:, kt, q0 : q0 + QCH]
            keep2 = bass.AP(
                tensor=keep_slice.tensor,
                offset=keep_slice.offset,
                ap=[keep_slice.ap[0], [0, 2], keep_slice.ap[1]],
            )
            w2 = w.rearrange("p (r q) -> p r q", r=2)
            nc.vector.tensor_mul(w2, w2, keep2)
            vpA, vpB = pair_tiles[pr][2], pair_tiles[pr][3]
            nc.tensor.matmul(
                accA, vpA[:, kt], w[:, 0:QCH],
                start=(kt == 0), stop=(kt == NKT - 1),
            )
            nc.tensor.matmul(
                accB, vpB[:, kt], w[:, QCH : 2 * QCH],
                start=(kt == 0), stop=(kt == NKT - 1),
            )
            if kt == NKT - 1:
                # build the staggered epilogue schedule for this (pr, qc)
                state = {}
                hA2, hB2, eqc = 2 * pr, 2 * pr + 1, qc

                def mk(fn):
                    return fn

                pending_ops += [
                    (i + 2, mk(lambda s=state, a=accA: s.__setitem__("bcA", ep_recip("A", a)))),
                    (i + 3, mk(lambda s=state, a=accB: s.__setitem__("bcB", ep_recip("B", a)))),
                    (i + 5, mk(lambda s=state, a=accA: s.__setitem__("aA", ep_copy("A", a)))),
                    (i + 6, mk(lambda s=state, h=hA2, q=eqc: ep_store(h, q, s["aA"], s["bcA"]))),
                    (i + 7, mk(lambda s=state, a=accB: s.__setitem__("aB", ep_copy("B", a)))),
                    (i + 8, mk(lambda s=state, h=hB2, q=eqc: ep_store(h, q, s["aB"], s["bcB"]))),
                ]
            while pending_ops and pending_ops[0][0] <= i:
                pending_ops.pop(0)[1]()
        for _, fn in pending_ops:
            fn()
    nc.finalize()
    return nc


_NC_CACHE = {}


def get_nc():
    if "nc" not in _NC_CACHE:
        _NC_CACHE["nc"] = build_nc()
    return _NC_CACHE["nc"]


def kernel(Q, K, V, mask, _trace=False, _tmpdir=None):
    Q = np.asarray(Q, dtype=np.float32)
    K = np.asarray(K, dtype=np.float32)
    V = np.asarray(V, dtype=np.float32)
    mask = np.asarray(mask)

    in_maps = []
    for c in range(N_CORES):
        b, h0 = c // 2, (c % 2) * HPC
        # [pair, {q,k}, 128, S]: partitions 0:64 = head A dims, 64:128 = head B
        qkt = np.empty((NPAIR, 2, P, S), BF)
        qt = Q[b, h0 : h0 + HPC].transpose(0, 2, 1).reshape(NPAIR, 2 * DK, S)
        kt = K[b, h0 : h0 + HPC].transpose(0, 2, 1).reshape(NPAIR, 2 * DK, S)
        qkt[:, 0] = qt
        qkt[:, 1] = kt
        vp = np.empty((HPC, P, NKT, DK + 1), BF)
        vp[:, :, :, 0:DK] = (
            V[b, h0 : h0 + HPC].reshape(HPC, NKT, P, DK).transpose(0, 2, 1, 3)
        )
        vp[:, :, :, DK] = 1.0
        if c % 2 == 0:
            kp = (~mask[b, 0]).T  # [k, q]
            keep = np.ascontiguousarray(
                kp.reshape(NKT, P, S).transpose(1, 0, 2)
            ).astype(BF)
        in_maps.append({"qkt": qkt, "vp": vp, "keep": keep})

    nc = get_nc()
    res = run_bass_kernel_spmd(
        nc, in_maps, core_ids=list(range(N_CORES)), trace=_trace, tmpdir=_tmpdir
    )
    out = np.empty((B, H, S, DK), np.float32)
    for c in range(N_CORES):
        b, h0 = c // 2, (c % 2) * HPC
        out[b, h0 : h0 + HPC] = np.asarray(res.results[c]["outT"]).transpose(0, 2, 1)
    if _trace:
        return out, res
    return out


# revision 29
# speedup vs baseline: 1.0886x; 1.0084x over previous
"""Sharded multi-head attention for TRN2 (8 NeuronCores).

Problem: B=4, H=16, S=2048, DK=64 attention with boolean mask [B,1,S,S]
(True entries masked out).  The 64 (batch, head) pairs are independent:
core c handles batch c//2, heads (c%2)*8 .. (c%2)*8+8.

Per-core algorithm, heads processed in PAIRS (A, B) sharing the PE array:
  - scores_T[k, q] for A and B run CONCURRENTLY in PE row-groups [0:64] /
    [64:128] (contraction dim d=64 each, tile_position packing).
  - one ACT exp over the pair's [128, 1024] PSUM tile (scale=1/8 folded,
    no max-subtraction: scores ~ N(0,1)).
  - mask multiply on DVE (bf16 2x), keep_T shared across heads.
  - PV: acc[v, q] += V'[k, v]^T w per head, V' = [V | ones] so row 64 of
    acc accumulates the softmax denominators.
  - epilogue per (head, q-chunk): reciprocal_approx_fast of row 64,
    rank-1 PE broadcast to 64 partitions, multiply, DMA out in [d, q]
    layout (host un-transposes the view; pure layout, no host math).

All DMAs are partition-major with >=2KB contiguous runs (host pre-swizzles
inputs, ones column baked into V').
"""

import numpy as np
import ml_dtypes
from contextlib import ExitStack

import concourse.bass as bass
import concourse.tile as tile
from concourse import bacc, mybir
from concourse.bass_utils import run_bass_kernel_spmd

B, H, S, DK = 4, 16, 2048, 64
N_CORES = 8
HPC = (B * H) // N_CORES  # heads per core = 8
NPAIR = HPC // 2

P = 128            # k-tile size / partition count
NKT = S // P       # 16 k tiles
QCH = 512          # q chunk per head (pair tile = [128, 1024] = 2 PSUM banks)
NQ = S // QCH      # 4 q chunks

BF16 = mybir.dt.bfloat16
F32 = mybir.dt.float32
BF = ml_dtypes.bfloat16


def build_nc():
    nc = bacc.Bacc(None, target_bir_lowering=False)
    # qkt[pair, 0] = [Q_A^T ; Q_B^T] stacked on partitions, [pair, 1] = K
    qkt_ext = nc.declare_dram_parameter("qkt", [NPAIR, 2, P, S], BF16, isOutput=False)
    # vp[h, p, t, :] = [V[h, t*128+p, :], 1.0]
    vp_ext = nc.declare_dram_parameter("vp", [HPC, P, NKT, DK + 1], BF16, isOutput=False)
    # keep[p, t, q] = not mask[q, t*128+p]
    keep_ext = nc.declare_dram_parameter("keep", [P, NKT, S], BF16, isOutput=False)
    # out_T[h, d, q] (host un-transposes)
    out_ext = nc.declare_dram_parameter("outT", [HPC, DK, S], F32, isOutput=True)

    with tile.TileContext(nc) as tc, ExitStack() as ctx:
        singles = ctx.enter_context(tc.tile_pool(name="singles", bufs=1))
        qk_pool = ctx.enter_context(tc.tile_pool(name="qk", bufs=2))
        v_pool = ctx.enter_context(tc.tile_pool(name="vpool", bufs=2))
        w_pool = ctx.enter_context(tc.tile_pool(name="wp", bufs=8))
        ep_pool = ctx.enter_context(tc.tile_pool(name="ep", bufs=2))
        sc_ps = ctx.enter_context(tc.tile_pool(name="scps", bufs=2, space="PSUM"))
        acc_ps = ctx.enter_context(tc.tile_pool(name="accps", bufs=2, space="PSUM"))

        keep_sb = singles.tile([P, NKT, S], BF16)
        for kt in range(NKT):
            nc.gpsimd.dma_start(out=keep_sb[:, kt], in_=keep_ext[:, kt])

        # ---- flat software pipeline over (pair, qc, kt) ----
        pair_tiles = {}

        def get_pair(pair):
            if pair not in pair_tiles:
                hA, hB = 2 * pair, 2 * pair + 1
                qT2 = qk_pool.tile([P, S], BF16, tag="qT2", name=f"qT2_{pair}")
                kT2 = qk_pool.tile([P, S], BF16, tag="kT2", name=f"kT2_{pair}")
                nc.sync.dma_start(out=qT2, in_=qkt_ext[pair, 0])
                nc.sync.dma_start(out=kT2, in_=qkt_ext[pair, 1])
                vpA = v_pool.tile([P, NKT, DK + 1], BF16, tag="vpA", name=f"vpA_{pair}")
                vpB = v_pool.tile([P, NKT, DK + 1], BF16, tag="vpB", name=f"vpB_{pair}")
                nc.sync.dma_start(out=vpA, in_=vp_ext[hA])
                nc.sync.dma_start(out=vpB, in_=vp_ext[hB])
                pair_tiles[pair] = (qT2, kT2, vpA, vpB)
            return pair_tiles[pair]

        def issue_qk(pair, qc, kt):
            qT2, kT2, _, _ = get_pair(pair)
            q0, k0 = qc * QCH, kt * P
            sc = sc_ps.tile([P, 2 * QCH], F32, tag="sc", name=f"sc_{pair}_{qc}_{kt}")
            nc.tensor.matmul(
                sc[:, 0:QCH],
                kT2[0:DK, k0 : k0 + P],
                qT2[0:DK, q0 : q0 + QCH],
                start=True,
                stop=True,
                tile_position=(0, 0),
            )
            nc.tensor.matmul(
                sc[:, QCH : 2 * QCH],
                kT2[DK : 2 * DK, k0 : k0 + P],
                qT2[DK : 2 * DK, q0 : q0 + QCH],
                start=True,
                stop=True,
                tile_position=(64, 0),
            )
            return sc

        def ep_recip(tag, acc):
            """reciprocal of the sums row (DVE, reads PSUM)"""
            rowF = ep_pool.tile([1, QCH], F32, tag=f"rowF{tag}")
            nc.vector.tensor_copy(rowF, acc[DK : DK + 1, :])
            recipF = ep_pool.tile([1, QCH], F32, tag=f"recipF{tag}")
            nc.vector.reciprocal_approx_fast(recipF, rowF)
            recipS = ep_pool.tile([1, QCH], BF16, tag=f"recipS{tag}")
            nc.vector.tensor_copy(recipS, recipF)
            bcS = ep_pool.tile([DK, QCH], BF16, tag=f"bcS{tag}")
            nc.gpsimd.partition_broadcast(bcS, recipS)
            return bcS

        def ep_copy(tag, acc):
            """PSUM -> SBUF copy of the accumulator (DVE)"""
            accS = ep_pool.tile([DK + 1, QCH], BF16, tag=f"accS{tag}")
            nc.vector.tensor_copy(accS, acc)
            return accS

        def ep_store(h, qc, accS, bcS):
            """normalize + store"""
            q0 = qc * QCH
            outf = ep_pool.tile([DK, QCH], F32, tag="outf")
            nc.vector.tensor_mul(outf, accS[0:DK], bcS)
            nc.gpsimd.dma_start(out=out_ext[h, :, q0 : q0 + QCH], in_=outf)

        iters = [
            (pr, qc, kt)
            for pr in range(NPAIR)
            for qc in range(NQ)
            for kt in range(NKT)
        ]
        # epilogue micro-ops deferred past their producers; (due_iter, fn)
        pending_ops = []
        # PVs are issued PV_LAG iterations late so a PV never sits at the
        # head of the in-order PE queue waiting on a just-computed mask
        # multiply (which would park the next QKs behind it)
        PV_LAG = 2
        pending_pv = []
        accs = None
        sc = issue_qk(*iters[0])
        for i, (pr, qc, kt) in enumerate(iters):
            if kt == 0:
                accs = (
                    acc_ps.tile([DK + 1, QCH], F32, tag="accA", name=f"accA_{pr}_{qc}"),
                    acc_ps.tile([DK + 1, QCH], F32, tag="accB", name=f"accB_{pr}_{qc}"),
                )
            accA, accB = accs
            w = w_pool.tile([P, 2 * QCH], BF16, tag="w")
            nc.scalar.activation(w, sc, mybir.ActivationFunctionType.Exp, scale=0.125)
            # next iteration's QK issues BEFORE this iteration's PV so the
            # in-order PE queue never parks QK behind a PV still waiting on
            # exp/mask; crosses qc/pair boundaries
            if i + 1 < len(iters):
                sc = issue_qk(*iters[i + 1])
            # one masked multiply over both heads: keep slice broadcast
            # (stride-0) over the head dim
            q0 = qc * QCH
            keep_slice = keep_sb[:, kt, q0 : q0 + QCH]
            keep2 = bass.AP(
                tensor=keep_slice.tensor,
                offset=keep_slice.offset,
                ap=[keep_slice.ap[0], [0, 2], keep_slice.ap[1]],
            )
            w2 = w.rearrange("p (r q) -> p r q", r=2)
            nc.vector.tensor_mul(w2, w2, keep2)
            vpA, vpB = pair_tiles[pr][2], pair_tiles[pr][3]
            pending_pv.append((kt, w, accA, accB, vpA, vpB))
            if len(pending_pv) > PV_LAG:
                pkt, pw, pA, pB, pvA, pvB = pending_pv.pop(0)
                nc.tensor.matmul(
                    pA, pvA[:, pkt], pw[:, 0:QCH],
                    start=(pkt == 0), stop=(pkt == NKT - 1),
                )
                nc.tensor.matmul(
                    pB, pvB[:, pkt], pw[:, QCH : 2 * QCH],
                    start=(pkt == 0), stop=(pkt == NKT - 1),
                )
            if kt == NKT - 1:
                # epilogue schedule for this (pr, qc): recips after the
                # (lagged) last PV, normalize/store after the broadcast
                state = {}
                hA2, hB2, eqc = 2 * pr, 2 * pr + 1, qc

                def p1(s=state, a=accA, b=accB):
                    s["bcA"] = ep_recip("A", a)
                    s["bcB"] = ep_recip("B", b)

                def p2(s=state, a=accA, b=accB, ha=hA2, hb=hB2, q=eqc):
                    aA = ep_copy("A", a)
                    ep_store(ha, q, aA, s["bcA"])
                    aB = ep_copy("B", b)
                    ep_store(hb, q, aB, s["bcB"])

                pending_ops += [(i + PV_LAG + 2, p1), (i + PV_LAG + 6, p2)]
            while pending_ops and pending_ops[0][0] <= i:
                pending_ops.pop(0)[1]()
        for pkt, pw, pA, pB, pvA, pvB in pending_pv:
            nc.tensor.matmul(
                pA, pvA[:, pkt], pw[:, 0:QCH],
                start=(pkt == 0), stop=(pkt == NKT - 1),
            )
            nc.tensor.matmul(
                pB, pvB[:, pkt], pw[:, QCH : 2 * QCH],
                start=(pkt == 0), stop=(pkt == NKT - 1),
            )
        for _, fn in pending_ops:
            fn()
    nc.finalize()
    return nc


_NC_CACHE = {}


def get_nc():
    if "nc" not in _NC_CACHE:
        _NC_CACHE["nc"] = build_nc()
    return _NC_CACHE["nc"]


def kernel(Q, K, V, mask, _trace=False, _tmpdir=None):
    Q = np.asarray(Q, dtype=np.float32)
    K = np.asarray(K, dtype=np.float32)
    V = np.asarray(V, dtype=np.float32)
    mask = np.asarray(mask)

    in_maps = []
    for c in range(N_CORES):
        b, h0 = c // 2, (c % 2) * HPC
        # [pair, {q,k}, 128, S]: partitions 0:64 = head A dims, 64:128 = head B
        qkt = np.empty((NPAIR, 2, P, S), BF)
        qt = Q[b, h0 : h0 + HPC].transpose(0, 2, 1).reshape(NPAIR, 2 * DK, S)
        kt = K[b, h0 : h0 + HPC].transpose(0, 2, 1).reshape(NPAIR, 2 * DK, S)
        qkt[:, 0] = qt
        qkt[:, 1] = kt
        vp = np.empty((HPC, P, NKT, DK + 1), BF)
        vp[:, :, :, 0:DK] = (
            V[b, h0 : h0 + HPC].reshape(HPC, NKT, P, DK).transpose(0, 2, 1, 3)
        )
        vp[:, :, :, DK] = 1.0
        if c % 2 == 0:
            kp = (~mask[b, 0]).T  # [k, q]
            keep = np.ascontiguousarray(
                kp.reshape(NKT, P, S).transpose(1, 0, 2)
            ).astype(BF)
        in_maps.append({"qkt": qkt, "vp": vp, "keep": keep})

    nc = get_nc()
    res = run_bass_kernel_spmd(
        nc, in_maps, core_ids=list(range(N_CORES)), trace=_trace, tmpdir=_tmpdir
    )
    out = np.empty((B, H, S, DK), np.float32)
    for c in range(N_CORES):
        b, h0 = c // 2, (c % 2) * HPC
        out[b, h0 : h0 + HPC] = np.asarray(res.results[c]["outT"]).transpose(0, 2, 1)
    if _trace:
        return out, res
    return out


# revision 31
# speedup vs baseline: 1.1498x; 1.0561x over previous
"""Sharded multi-head attention for TRN2 (8 NeuronCores).

Problem: B=4, H=16, S=2048, DK=64 attention with boolean mask [B,1,S,S]
(True entries masked out).  The 64 (batch, head) pairs are independent:
core c handles batch c//2, heads (c%2)*8 .. (c%2)*8+8.

Per-core algorithm, heads processed in PAIRS (A, B) sharing the PE array:
  - scores_T[k, q] for A and B run CONCURRENTLY in PE row-groups [0:64] /
    [64:128] (contraction dim d=64 each, tile_position packing).
  - one ACT exp over the pair's [128, 1024] PSUM tile (scale=1/8 folded,
    no max-subtraction: scores ~ N(0,1)).
  - mask multiply on DVE (bf16 2x), keep_T shared across heads.
  - PV: acc[v, q] += V'[k, v]^T w per head, V' = [V | ones] so row 64 of
    acc accumulates the softmax denominators.
  - epilogue per (head, q-chunk): reciprocal_approx_fast of row 64,
    rank-1 PE broadcast to 64 partitions, multiply, DMA out in [d, q]
    layout (host un-transposes the view; pure layout, no host math).

All DMAs are partition-major with >=2KB contiguous runs (host pre-swizzles
inputs, ones column baked into V').
"""

import numpy as np
import ml_dtypes
from contextlib import ExitStack

import concourse.bass as bass
import concourse.tile as tile
from concourse import bacc, mybir
from concourse.bass_utils import run_bass_kernel_spmd

B, H, S, DK = 4, 16, 2048, 64
N_CORES = 8
HPC = (B * H) // N_CORES  # heads per core = 8
NPAIR = HPC // 2

P = 128            # k-tile size / partition count
NKT = S // P       # 16 k tiles
QCH = 512          # q chunk per head (pair tile = [128, 1024] = 2 PSUM banks)
NQ = S // QCH      # 4 q chunks

BF16 = mybir.dt.bfloat16
F32 = mybir.dt.float32
BF = ml_dtypes.bfloat16


def build_nc():
    nc = bacc.Bacc(None, target_bir_lowering=False)
    # qkt[pair, 0] = [Q_A^T ; Q_B^T] stacked on partitions, [pair, 1] = K
    qkt_ext = nc.declare_dram_parameter("qkt", [NPAIR, 2, P, S], BF16, isOutput=False)
    # vp[h, p, t, :] = [V[h, t*128+p, :], 1.0]
    vp_ext = nc.declare_dram_parameter("vp", [HPC, P, NKT, DK + 1], BF16, isOutput=False)
    # keep[p, t, q] = not mask[q, t*128+p]
    keep_ext = nc.declare_dram_parameter("keep", [P, NKT, S], BF16, isOutput=False)
    # out_T[h, d, q] (host un-transposes)
    out_ext = nc.declare_dram_parameter("outT", [HPC, DK, S], F32, isOutput=True)

    with tile.TileContext(nc) as tc, ExitStack() as ctx:
        singles = ctx.enter_context(tc.tile_pool(name="singles", bufs=1))
        qk_pool = ctx.enter_context(tc.tile_pool(name="qk", bufs=2))
        v_pool = ctx.enter_context(tc.tile_pool(name="vpool", bufs=2))
        w_pool = ctx.enter_context(tc.tile_pool(name="wp", bufs=8))
        ep_pool = ctx.enter_context(tc.tile_pool(name="ep", bufs=2))
        sc_ps = ctx.enter_context(tc.tile_pool(name="scps", bufs=2, space="PSUM"))
        acc_ps = ctx.enter_context(tc.tile_pool(name="accps", bufs=2, space="PSUM"))

        keep_sb = singles.tile([P, NKT, S], BF16)
        for kt in range(NKT):
            nc.gpsimd.dma_start(out=keep_sb[:, kt], in_=keep_ext[:, kt])

        # ---- flat software pipeline over (pair, qc, kt) ----
        pair_tiles = {}

        def get_pair(pair):
            if pair not in pair_tiles:
                hA, hB = 2 * pair, 2 * pair + 1
                qT2 = qk_pool.tile([P, S], BF16, tag="qT2", name=f"qT2_{pair}")
                kT2 = qk_pool.tile([P, S], BF16, tag="kT2", name=f"kT2_{pair}")
                nc.sync.dma_start(out=qT2, in_=qkt_ext[pair, 0])
                nc.sync.dma_start(out=kT2, in_=qkt_ext[pair, 1])
                vpA = v_pool.tile([P, NKT, DK + 1], BF16, tag="vpA", name=f"vpA_{pair}")
                vpB = v_pool.tile([P, NKT, DK + 1], BF16, tag="vpB", name=f"vpB_{pair}")
                nc.sync.dma_start(out=vpA, in_=vp_ext[hA])
                nc.sync.dma_start(out=vpB, in_=vp_ext[hB])
                pair_tiles[pair] = (qT2, kT2, vpA, vpB)
            return pair_tiles[pair]

        def issue_qk(pair, qc, kt):
            qT2, kT2, _, _ = get_pair(pair)
            q0, k0 = qc * QCH, kt * P
            sc = sc_ps.tile([P, 2 * QCH], F32, tag="sc", name=f"sc_{pair}_{qc}_{kt}")
            nc.tensor.matmul(
                sc[:, 0:QCH],
                kT2[0:DK, k0 : k0 + P],
                qT2[0:DK, q0 : q0 + QCH],
                start=True,
                stop=True,
                tile_position=(0, 0),
            )
            nc.tensor.matmul(
                sc[:, QCH : 2 * QCH],
                kT2[DK : 2 * DK, k0 : k0 + P],
                qT2[DK : 2 * DK, q0 : q0 + QCH],
                start=True,
                stop=True,
                tile_position=(64, 0),
            )
            return sc

        def ep_recip(tag, acc):
            """reciprocal of the sums row (DVE, reads PSUM)"""
            rowF = ep_pool.tile([1, QCH], F32, tag=f"rowF{tag}")
            nc.vector.tensor_copy(rowF, acc[DK : DK + 1, :])
            recipF = ep_pool.tile([1, QCH], F32, tag=f"recipF{tag}")
            nc.vector.reciprocal_approx_fast(recipF, rowF)
            recipS = ep_pool.tile([1, QCH], BF16, tag=f"recipS{tag}")
            nc.vector.tensor_copy(recipS, recipF)
            bcS = ep_pool.tile([DK, QCH], BF16, tag=f"bcS{tag}")
            nc.gpsimd.partition_broadcast(bcS, recipS)
            return bcS

        def ep_copy(tag, acc):
            """PSUM -> SBUF copy of the accumulator (DVE)"""
            accS = ep_pool.tile([DK + 1, QCH], BF16, tag=f"accS{tag}")
            nc.vector.tensor_copy(accS, acc)
            return accS

        def ep_store(h, qc, accS, bcS):
            """normalize + store"""
            q0 = qc * QCH
            outf = ep_pool.tile([DK, QCH], F32, tag="outf")
            nc.vector.tensor_mul(outf, accS[0:DK], bcS)
            nc.gpsimd.dma_start(out=out_ext[h, :, q0 : q0 + QCH], in_=outf)

        iters = [
            (pr, qc, kt)
            for pr in range(NPAIR)
            for qc in range(NQ)
            for kt in range(NKT)
        ]
        # epilogue micro-ops deferred past their producers; (due_iter, fn)
        pending_ops = []
        # PVs are issued PV_LAG iterations late so a PV never sits at the
        # head of the in-order PE queue waiting on a just-computed mask
        # multiply (which would park the next QKs behind it)
        PV_LAG = 0
        pending_pv = []
        accs = None
        sc = issue_qk(*iters[0])
        for i, (pr, qc, kt) in enumerate(iters):
            if kt == 0:
                accs = (
                    acc_ps.tile([DK + 1, QCH], F32, tag="accA", name=f"accA_{pr}_{qc}"),
                    acc_ps.tile([DK + 1, QCH], F32, tag="accB", name=f"accB_{pr}_{qc}"),
                )
            accA, accB = accs
            w = w_pool.tile([P, 2 * QCH], BF16, tag="w")
            nc.scalar.activation(w, sc, mybir.ActivationFunctionType.Exp, scale=0.125)
            # next iteration's QK issues BEFORE this iteration's PV so the
            # in-order PE queue never parks QK behind a PV still waiting on
            # exp/mask; crosses qc/pair boundaries
            if i + 1 < len(iters):
                sc = issue_qk(*iters[i + 1])
            # one masked multiply over both heads: keep slice broadcast
            # (stride-0) over the head dim
            q0 = qc * QCH
            keep_slice = keep_sb[:, kt, q0 : q0 + QCH]
            keep2 = bass.AP(
                tensor=keep_slice.tensor,
                offset=keep_slice.offset,
                ap=[keep_slice.ap[0], [0, 2], keep_slice.ap[1]],
            )
            w2 = w.rearrange("p (r q) -> p r q", r=2)
            nc.vector.tensor_mul(w2, w2, keep2)
            vpA, vpB = pair_tiles[pr][2], pair_tiles[pr][3]
            pending_pv.append((kt, w, accA, accB, vpA, vpB))
            if len(pending_pv) > PV_LAG:
                pkt, pw, pA, pB, pvA, pvB = pending_pv.pop(0)
                nc.tensor.matmul(
                    pA, pvA[:, pkt], pw[:, 0:QCH],
                    start=(pkt == 0), stop=(pkt == NKT - 1),
                )
                nc.tensor.matmul(
                    pB, pvB[:, pkt], pw[:, QCH : 2 * QCH],
                    start=(pkt == 0), stop=(pkt == NKT - 1),
                )
            if kt == NKT - 1:
                # epilogue schedule for this (pr, qc): recips after the
                # (lagged) last PV, normalize/store after the broadcast
                state = {}
                hA2, hB2, eqc = 2 * pr, 2 * pr + 1, qc

                def p1(s=state, a=accA, b=accB):
                    s["bcA"] = ep_recip("A", a)
                    s["bcB"] = ep_recip("B", b)

                def p2(s=state, a=accA, b=accB, ha=hA2, hb=hB2, q=eqc):
                    aA = ep_copy("A", a)
                    ep_store(ha, q, aA, s["bcA"])
                    aB = ep_copy("B", b)
                    ep_store(hb, q, aB, s["bcB"])

                pending_ops += [(i + PV_LAG + 2, p1), (i + PV_LAG + 7, p2)]
            while pending_ops and pending_ops[0][0] <= i:
                pending_ops.pop(0)[1]()
        for pkt, pw, pA, pB, pvA, pvB in pending_pv:
            nc.tensor.matmul(
                pA, pvA[:, pkt], pw[:, 0:QCH],
                start=(pkt == 0), stop=(pkt == NKT - 1),
            )
            nc.tensor.matmul(
                pB, pvB[:, pkt], pw[:, QCH : 2 * QCH],
                start=(pkt == 0), stop=(pkt == NKT - 1),
            )
        for _, fn in pending_ops:
            fn()
    nc.finalize()
    return nc


_NC_CACHE = {}


def get_nc():
    if "nc" not in _NC_CACHE:
        _NC_CACHE["nc"] = build_nc()
    return _NC_CACHE["nc"]


def kernel(Q, K, V, mask, _trace=False, _tmpdir=None):
    Q = np.asarray(Q, dtype=np.float32)
    K = np.asarray(K, dtype=np.float32)
    V = np.asarray(V, dtype=np.float32)
    mask = np.asarray(mask)

    in_maps = []
    for c in range(N_CORES):
        b, h0 = c // 2, (c % 2) * HPC
        # [pair, {q,k}, 128, S]: partitions 0:64 = head A dims, 64:128 = head B
        qkt = np.empty((NPAIR, 2, P, S), BF)
        qt = Q[b, h0 : h0 + HPC].transpose(0, 2, 1).reshape(NPAIR, 2 * DK, S)
        kt = K[b, h0 : h0 + HPC].transpose(0, 2, 1).reshape(NPAIR, 2 * DK, S)
        qkt[:, 0] = qt
        qkt[:, 1] = kt
        vp = np.empty((HPC, P, NKT, DK + 1), BF)
        vp[:, :, :, 0:DK] = (
            V[b, h0 : h0 + HPC].reshape(HPC, NKT, P, DK).transpose(0, 2, 1, 3)
        )
        vp[:, :, :, DK] = 1.0
        if c % 2 == 0:
            kp = (~mask[b, 0]).T  # [k, q]
            keep = np.ascontiguousarray(
                kp.reshape(NKT, P, S).transpose(1, 0, 2)
            ).astype(BF)
        in_maps.append({"qkt": qkt, "vp": vp, "keep": keep})

    nc = get_nc()
    res = run_bass_kernel_spmd(
        nc, in_maps, core_ids=list(range(N_CORES)), trace=_trace, tmpdir=_tmpdir
    )
    out = np.empty((B, H, S, DK), np.float32)
    for c in range(N_CORES):
        b, h0 = c // 2, (c % 2) * HPC
        out[b, h0 : h0 + HPC] = np.asarray(res.results[c]["outT"]).transpose(0, 2, 1)
    if _trace:
        return out, res
    return out


# revision 33
# speedup vs baseline: 1.1613x; 1.0101x over previous
"""Sharded multi-head attention for TRN2 (8 NeuronCores).

Problem: B=4, H=16, S=2048, DK=64 attention with boolean mask [B,1,S,S]
(True entries masked out).  The 64 (batch, head) pairs are independent:
core c handles batch c//2, heads (c%2)*8 .. (c%2)*8+8.

Per-core algorithm, heads processed in PAIRS (A, B) sharing the PE array:
  - scores_T[k, q] for A and B run CONCURRENTLY in PE row-groups [0:64] /
    [64:128] (contraction dim d=64 each, tile_position packing).
  - one ACT exp over the pair's [128, 1024] PSUM tile (scale=1/8 folded,
    no max-subtraction: scores ~ N(0,1)).
  - mask multiply on DVE (bf16 2x), keep_T shared across heads.
  - PV: acc[v, q] += V'[k, v]^T w per head, V' = [V | ones] so row 64 of
    acc accumulates the softmax denominators.
  - epilogue per (head, q-chunk): reciprocal_approx_fast of row 64,
    rank-1 PE broadcast to 64 partitions, multiply, DMA out in [d, q]
    layout (host un-transposes the view; pure layout, no host math).

All DMAs are partition-major with >=2KB contiguous runs (host pre-swizzles
inputs, ones column baked into V').
"""

import numpy as np
import ml_dtypes
from contextlib import ExitStack

import concourse.bass as bass
import concourse.tile as tile
from concourse import bacc, mybir
from concourse.bass_utils import run_bass_kernel_spmd

B, H, S, DK = 4, 16, 2048, 64
N_CORES = 8
HPC = (B * H) // N_CORES  # heads per core = 8
NPAIR = HPC // 2

P = 128            # k-tile size / partition count
NKT = S // P       # 16 k tiles
QCH = 512          # q chunk per head (pair tile = [128, 1024] = 2 PSUM banks)
NQ = S // QCH      # 4 q chunks

BF16 = mybir.dt.bfloat16
F32 = mybir.dt.float32
BF = ml_dtypes.bfloat16


def build_nc():
    nc = bacc.Bacc(None, target_bir_lowering=False)
    # qkt[pair, 0] = [Q_A^T ; Q_B^T] stacked on partitions, [pair, 1] = K
    qkt_ext = nc.declare_dram_parameter("qkt", [NPAIR, 2, P, S], BF16, isOutput=False)
    # vp[h, p, t, :] = [V[h, t*128+p, :], 1.0]
    vp_ext = nc.declare_dram_parameter("vp", [HPC, P, NKT, DK + 1], BF16, isOutput=False)
    # keep[p, t, q] = not mask[q, t*128+p]
    keep_ext = nc.declare_dram_parameter("keep", [P, NKT, S], BF16, isOutput=False)
    # out_T[h, d, q] (host un-transposes)
    out_ext = nc.declare_dram_parameter("outT", [HPC, DK, S], F32, isOutput=True)

    with tile.TileContext(nc) as tc, ExitStack() as ctx:
        singles = ctx.enter_context(tc.tile_pool(name="singles", bufs=1))
        qk_pool = ctx.enter_context(tc.tile_pool(name="qk", bufs=2))
        v_pool = ctx.enter_context(tc.tile_pool(name="vpool", bufs=2))
        w_pool = ctx.enter_context(tc.tile_pool(name="wp", bufs=8))
        ep_pool = ctx.enter_context(tc.tile_pool(name="ep", bufs=2))
        sc_ps = ctx.enter_context(tc.tile_pool(name="scps", bufs=2, space="PSUM"))
        acc_ps = ctx.enter_context(tc.tile_pool(name="accps", bufs=2, space="PSUM"))

        keep_sb = singles.tile([P, NKT, S], BF16)
        for kt in range(NKT):
            nc.gpsimd.dma_start(out=keep_sb[:, kt], in_=keep_ext[:, kt])

        # ---- flat software pipeline over (pair, qc, kt) ----
        pair_tiles = {}

        def get_pair(pair):
            if pair not in pair_tiles:
                hA, hB = 2 * pair, 2 * pair + 1
                qT2 = qk_pool.tile([P, S], BF16, tag="qT2", name=f"qT2_{pair}")
                kT2 = qk_pool.tile([P, S], BF16, tag="kT2", name=f"kT2_{pair}")
                nc.sync.dma_start(out=qT2, in_=qkt_ext[pair, 0])
                nc.sync.dma_start(out=kT2, in_=qkt_ext[pair, 1])
                vpA = v_pool.tile([P, NKT, DK + 1], BF16, tag="vpA", name=f"vpA_{pair}")
                vpB = v_pool.tile([P, NKT, DK + 1], BF16, tag="vpB", name=f"vpB_{pair}")
                nc.sync.dma_start(out=vpA, in_=vp_ext[hA])
                nc.sync.dma_start(out=vpB, in_=vp_ext[hB])
                pair_tiles[pair] = (qT2, kT2, vpA, vpB)
            return pair_tiles[pair]

        def issue_qk(pair, qc, kt):
            qT2, kT2, _, _ = get_pair(pair)
            q0, k0 = qc * QCH, kt * P
            sc = sc_ps.tile([P, 2 * QCH], F32, tag="sc", name=f"sc_{pair}_{qc}_{kt}")
            nc.tensor.matmul(
                sc[:, 0:QCH],
                kT2[0:DK, k0 : k0 + P],
                qT2[0:DK, q0 : q0 + QCH],
                start=True,
                stop=True,
                tile_position=(0, 0),
            )
            nc.tensor.matmul(
                sc[:, QCH : 2 * QCH],
                kT2[DK : 2 * DK, k0 : k0 + P],
                qT2[DK : 2 * DK, q0 : q0 + QCH],
                start=True,
                stop=True,
                tile_position=(64, 0),
            )
            return sc

        def ep_recip(tag, acc):
            """reciprocal of the sums row (DVE, reads PSUM)"""
            rowF = ep_pool.tile([1, QCH], F32, tag=f"rowF{tag}")
            nc.vector.tensor_copy(rowF, acc[DK : DK + 1, :])
            recipF = ep_pool.tile([1, QCH], F32, tag=f"recipF{tag}")
            nc.vector.reciprocal_approx_fast(recipF, rowF)
            recipS = ep_pool.tile([1, QCH], BF16, tag=f"recipS{tag}")
            nc.vector.tensor_copy(recipS, recipF)
            bcS = ep_pool.tile([DK, QCH], BF16, tag=f"bcS{tag}")
            nc.gpsimd.partition_broadcast(bcS, recipS)
            return bcS

        def ep_store(h, qc, acc, bcS):
            """normalize straight out of PSUM + store (one fused TT)"""
            q0 = qc * QCH
            outf = ep_pool.tile([DK, QCH], F32, tag="outf")
            nc.vector.tensor_mul(outf, acc[0:DK], bcS)
            nc.gpsimd.dma_start(out=out_ext[h, :, q0 : q0 + QCH], in_=outf)

        iters = [
            (pr, qc, kt)
            for pr in range(NPAIR)
            for qc in range(NQ)
            for kt in range(NKT)
        ]
        # epilogue micro-ops deferred past their producers; (due_iter, fn)
        pending_ops = []
        # PVs are issued PV_LAG iterations late so a PV never sits at the
        # head of the in-order PE queue waiting on a just-computed mask
        # multiply (which would park the next QKs behind it)
        PV_LAG = 0
        pending_pv = []
        accs = None
        sc = issue_qk(*iters[0])
        for i, (pr, qc, kt) in enumerate(iters):
            if kt == 0:
                accs = (
                    acc_ps.tile([DK + 1, QCH], F32, tag="accA", name=f"accA_{pr}_{qc}"),
                    acc_ps.tile([DK + 1, QCH], F32, tag="accB", name=f"accB_{pr}_{qc}"),
                )
            accA, accB = accs
            w = w_pool.tile([P, 2 * QCH], BF16, tag="w")
            nc.scalar.activation(w, sc, mybir.ActivationFunctionType.Exp, scale=0.125)
            # next iteration's QK issues BEFORE this iteration's PV so the
            # in-order PE queue never parks QK behind a PV still waiting on
            # exp/mask; crosses qc/pair boundaries
            if i + 1 < len(iters):
                sc = issue_qk(*iters[i + 1])
            # one masked multiply over both heads: keep slice broadcast
            # (stride-0) over the head dim
            q0 = qc * QCH
            keep_slice = keep_sb[:, kt, q0 : q0 + QCH]
            keep2 = bass.AP(
                tensor=keep_slice.tensor,
                offset=keep_slice.offset,
                ap=[keep_slice.ap[0], [0, 2], keep_slice.ap[1]],
            )
            w2 = w.rearrange("p (r q) -> p r q", r=2)
            nc.vector.tensor_mul(w2, w2, keep2)
            vpA, vpB = pair_tiles[pr][2], pair_tiles[pr][3]
            pending_pv.append((kt, w, accA, accB, vpA, vpB))
            if len(pending_pv) > PV_LAG:
                pkt, pw, pA, pB, pvA, pvB = pending_pv.pop(0)
                nc.tensor.matmul(
                    pA, pvA[:, pkt], pw[:, 0:QCH],
                    start=(pkt == 0), stop=(pkt == NKT - 1),
                )
                nc.tensor.matmul(
                    pB, pvB[:, pkt], pw[:, QCH : 2 * QCH],
                    start=(pkt == 0), stop=(pkt == NKT - 1),
                )
            if kt == NKT - 1:
                # epilogue schedule for this (pr, qc): recips after the
                # (lagged) last PV, normalize/store after the broadcast
                state = {}
                hA2, hB2, eqc = 2 * pr, 2 * pr + 1, qc

                def p1a(s=state, a=accA):
                    s["bcA"] = ep_recip("A", a)

                def p1b(s=state, b=accB):
                    s["bcB"] = ep_recip("B", b)

                def p2(s=state, a=accA, b=accB, ha=hA2, hb=hB2, q=eqc):
                    ep_store(ha, q, a, s["bcA"])
                    ep_store(hb, q, b, s["bcB"])

                pending_ops += [
                    (i + PV_LAG + 2, p1a),
                    (i + PV_LAG + 4, p1b),
                    (i + PV_LAG + 7, p2),
                ]
            while pending_ops and pending_ops[0][0] <= i:
                pending_ops.pop(0)[1]()
        for pkt, pw, pA, pB, pvA, pvB in pending_pv:
            nc.tensor.matmul(
                pA, pvA[:, pkt], pw[:, 0:QCH],
                start=(pkt == 0), stop=(pkt == NKT - 1),
            )
            nc.tensor.matmul(
                pB, pvB[:, pkt], pw[:, QCH : 2 * QCH],
                start=(pkt == 0), stop=(pkt == NKT - 1),
            )
        for _, fn in pending_ops:
            fn()
    nc.finalize()
    return nc


_NC_CACHE = {}


def get_nc():
    if "nc" not in _NC_CACHE:
        _NC_CACHE["nc"] = build_nc()
    return _NC_CACHE["nc"]


def kernel(Q, K, V, mask, _trace=False, _tmpdir=None):
    Q = np.asarray(Q, dtype=np.float32)
    K = np.asarray(K, dtype=np.float32)
    V = np.asarray(V, dtype=np.float32)
    mask = np.asarray(mask)

    in_maps = []
    for c in range(N_CORES):
        b, h0 = c // 2, (c % 2) * HPC
        # [pair, {q,k}, 128, S]: partitions 0:64 = head A dims, 64:128 = head B
        qkt = np.empty((NPAIR, 2, P, S), BF)
        qt = Q[b, h0 : h0 + HPC].transpose(0, 2, 1).reshape(NPAIR, 2 * DK, S)
        kt = K[b, h0 : h0 + HPC].transpose(0, 2, 1).reshape(NPAIR, 2 * DK, S)
        qkt[:, 0] = qt
        qkt[:, 1] = kt
        vp = np.empty((HPC, P, NKT, DK + 1), BF)
        vp[:, :, :, 0:DK] = (
            V[b, h0 : h0 + HPC].reshape(HPC, NKT, P, DK).transpose(0, 2, 1, 3)
        )
        vp[:, :, :, DK] = 1.0
        if c % 2 == 0:
            kp = (~mask[b, 0]).T  # [k, q]
            keep = np.ascontiguousarray(
                kp.reshape(NKT, P, S).transpose(1, 0, 2)
            ).astype(BF)
        in_maps.append({"qkt": qkt, "vp": vp, "keep": keep})

    nc = get_nc()
    res = run_bass_kernel_spmd(
        nc, in_maps, core_ids=list(range(N_CORES)), trace=_trace, tmpdir=_tmpdir
    )
    out = np.empty((B, H, S, DK), np.float32)
    for c in range(N_CORES):
        b, h0 = c // 2, (c % 2) * HPC
        out[b, h0 : h0 + HPC] = np.asarray(res.results[c]["outT"]).transpose(0, 2, 1)
    if _trace:
        return out, res
    return out
